# revision 58
# baseline (speedup 1.0000x reference)
"""Positional embedding lookup kernel for Trainium2 (8 NeuronCores).

Problem: out[b, t, :] = tok_weight[x[b, t], :] + pos_weight[t, :]
  x:          [4, 4096]  int32/int64 token ids in [0, 32000)
  tok_weight: [32000, 512] f32
  pos_weight: [4096, 512]  f32
  out:        [4, 4096, 512] f32

Sharding: split the 4096 positions into 8 contiguous chunks of 512; core c
handles positions [c*512, (c+1)*512) for ALL 4 batches (2048 tokens).  This
makes each core read only its 1MB slice of pos_weight (reused across the 4
batches) instead of a per-token 4MB read.

Per-core flat token order: i = 0..2047 walks (b, q) = (i//512, i%512),
i.e. flat_idx = x[:, c*512:(c+1)*512].ravel().  The gather lands token i at
SBUF partition i%128, column-block i//128, so column block col corresponds
to batch col//4, position sub-block col%4 — which aligns a whole batch's
512 tokens with the (identically laid out) pos tile for a single wide add.

The row gather uses the GPSIMD dma_gather custom op (one descriptor per
row, ~0.34ns/descriptor generation): 4 chunks of 512 rows, each split
into two 256-row gathers alternating across 2 SWDGE queues, so gather,
add, and store pipeline; indices are int16 (vocab 32000 < 32768), packed
i -> [i%16, i//16] over 16 partitions and replicated across the 8 Q7 cores.
"""

import numpy as np

import concourse.bass as bass
import concourse.tile as tile
from concourse import library_config, mybir
from concourse.bass_utils import run_bass_kernel_spmd

B = 4
T = 4096
E = 512
VOCAB = 32000
N_CORES = 8
POS_PER_CORE = T // N_CORES          # 512
TOK_PER_CORE = B * POS_PER_CORE      # 2048
P = 128
N_TILES = TOK_PER_CORE // P          # 16 column blocks of 128 tokens
JQ = POS_PER_CORE // P               # 4 pos sub-blocks
CHUNKS = 4                           # one gather/add/store chunk per batch
TOK_PER_CHUNK = TOK_PER_CORE // CHUNKS   # 512
IDX_COLS = TOK_PER_CORE // 16        # 128 int16 idx columns
SORTED_MODE = False                  # host-sorted gather rows (see make_in_maps)

_CACHE = {}


def _split_multi_waits(nc: bass.Bass) -> None:
    """Walrus codegen allows one sync-wait slot per TPB instruction (the
    NEURON_ISA_TPB_EVENTS struct); Tile can emit several.  Move extra waits
    onto standalone NoOps on the same engine, just before the instruction."""
    for func in nc.m.functions:
        for blk in func.blocks:
            new_insts = []
            for inst in blk.instructions:
                si = inst.sync_info
                if si is not None and len(si.on_wait) > 1:
                    for w in si.on_wait[:-1]:
                        nop = mybir.InstNoOp(
                            name=nc.get_next_instruction_name(),
                            engine=inst.engine,
                            bass_nofuse=True,
                            sync_info=mybir.SyncInfo(on_wait=[w], on_update=[]),
                        )
                        nc.register_instruction(nop)
                        new_insts.append(nop)
                    inst.sync_info = mybir.SyncInfo(
                        on_wait=si.on_wait[-1:], on_update=si.on_update
                    )
                new_insts.append(inst)
            blk.instructions[:] = new_insts


def _build_program(
    reps: int = 1,
    outer: int = 1,
    variant: str = "full",
    nqueues: int = 2,
    single_packet: bool = True,
    chunks: int = 4,
    out_part_major: bool = True,
    store_alt: bool = False,
    bufs: int = 3,
    split_gather: bool = True,
    sorted_mode: bool = False,
    gather_rows: int = 256,
) -> bass.Bass:
    """reps>1 unrolls the steady-state gather/add/store loop; outer>1 wraps
    it in a runtime For_i loop.  Used for timing: the wall-time delta
    between two total rep counts isolates device time.  variant isolates
    pipeline stages for benching: "full" | "gather" | "store"."""
    nc = bass.Bass(num_swdge_queues=nqueues)

    xti = nc.declare_dram_parameter(
        "xti", [P, IDX_COLS], mybir.dt.int16, isOutput=False
    )
    # sorted_mode: pos is pre-permuted per token slot (2048 rows); else the
    # core's 512 shared position rows
    pos_rows = TOK_PER_CORE if sorted_mode else POS_PER_CORE
    pos = nc.declare_dram_parameter(
        "pos", [pos_rows, E], mybir.dt.float32, isOutput=False
    )
    tok = nc.declare_dram_parameter(
        "tok", [VOCAB, E], mybir.dt.float32, isOutput=False
    )
    out_shape = [P, N_TILES, E] if out_part_major else [N_TILES, P, E]
    out = nc.declare_dram_parameter("out", out_shape, mybir.dt.float32, isOutput=True)

    with tile.TileContext(nc) as tc:
        with (
            tc.tile_pool(name="const", bufs=1) as const_pool,
            tc.tile_pool(name="work", bufs=bufs) as work_pool,
        ):
            # dma_gather lives in the 'mlp' GPSIMD firmware library
            nc.gpsimd.load_library(library_config.mlp)

            xti_t = const_pool.tile([P, IDX_COLS], mybir.dt.int16)
            nc.sync.dma_start(out=xti_t[:], in_=xti[:])

            # one DMA: partition p, col block c holds pos[c*128 + p, :]
            pos_blocks = pos_rows // P
            pos_t = const_pool.tile([P, pos_blocks * E], mybir.dt.float32)
            nc.sync.dma_start(
                out=pos_t[:].rearrange("p (c e) -> p c e", c=pos_blocks),
                in_=pos.rearrange("(c p) e -> p c e", p=P),
            )
            # tiny DVE op so the vector engine observes the const-load DMA
            # semaphores once; later adds then need only the gather wait.
            obs = const_pool.tile([P, 1], mybir.dt.float32, tag="obs")
            nc.vector.tensor_copy(out=obs[:], in_=pos_t[:, 0:1])

            # chunks: int (uniform) or list of per-chunk column-block counts
            # (a tapered schedule shortens pipeline fill and drain tail)
            if isinstance(chunks, int):
                assert chunks in (1, 2, 4, 8, 16)
                sched = [N_TILES // chunks] * chunks
            else:
                sched = list(chunks)
                assert sum(sched) == N_TILES and all(
                    n in (1, 2, 4, 8, 16) for n in sched
                )
            starts = [sum(sched[:i]) for i in range(len(sched))]
            _pb = max(gather_rows // P, 1)
            rows_needed = set()
            for n in set(sched):
                step = _pb if split_gather else n
                off = 0
                while off < n:
                    m = min(step, n - off)
                    rows_needed.add(m * P)
                    off += m
            nidx_regs = {r: nc.gpsimd.to_reg(r) for r in sorted(rows_needed)}
            # gather piece size in column blocks (256 rows = 2 blocks is the
            # measured read sweet spot; no split if split_gather=False)
            piece_blocks = max(gather_rows // P, 1)
            ib = IDX_COLS // N_TILES             # idx columns per block (8)

            def gather_into(g, s, n, qbase):
                """Gather col-blocks [s, s+n) of the rep into tile g."""
                step = piece_blocks if split_gather else n
                off, q = 0, qbase
                while off < n:
                    m = min(step, n - off)
                    nc.gpsimd.dma_gather(
                        g[:, off * E : (off + m) * E].rearrange(
                            "p (c e) -> p c e", e=E
                        ),
                        tok[:],
                        xti_t[:, (s + off) * ib : (s + off + m) * ib],
                        m * P,
                        nidx_regs[m * P],
                        E,
                        single_packet=single_packet,
                        queue_num=q % nqueues,
                    )
                    off += m
                    q += 1
                return q

            def add_pos(g, s, n):
                if sorted_mode:
                    # pos_t is slot-aligned: one add per chunk
                    nc.vector.tensor_add(
                        out=g[:, : n * E],
                        in0=g[:, : n * E],
                        in1=pos_t[:, s * E : (s + n) * E],
                    )
                    return
                # pos pattern repeats every JQ column blocks
                w = min(n, JQ)
                jq0 = s % JQ
                assert jq0 + w <= JQ, (s, n)
                in1 = pos_t[:, jq0 * E : (jq0 + w) * E]
                for h in range(0, n * E, w * E):
                    nc.vector.tensor_add(
                        out=g[:, h : h + w * E], in0=g[:, h : h + w * E], in1=in1
                    )

            g0 = None
            if variant == "store":
                n0 = sched[0]
                g0 = const_pool.tile([P, n0 * E], mybir.dt.float32, tag="g0")
                gather_into(g0, 0, n0, 0)
                add_pos(g0, 0, n0)

            def body():
                qi = 0
                for _ in range(reps):
                    for s, n in zip(starts, sched):
                        if variant == "store":
                            g, n = g0, sched[0]
                            s = min(s, N_TILES - n)
                        else:
                            g = work_pool.tile(
                                [P, max(sched) * E], mybir.dt.float32, tag="work"
                            )
                            qi = gather_into(g, s, n, qi)
                        if variant == "full":
                            add_pos(g, s, n)
                        if variant in ("full", "store", "noadd"):
                            if out_part_major:
                                out_ap = out[:, s : s + n, :]
                            else:
                                out_ap = out[s : s + n].rearrange("c p e -> p c e")
                            st_eng = (
                                nc.scalar if (store_alt and s % 2) else nc.sync
                            )
                            st_eng.dma_start(
                                out=out_ap,
                                in_=g[:, : n * E].rearrange(
                                    "p (c e) -> p c e", e=E
                                ),
                            )

            if outer > 1:
                with tc.For_i(0, outer):
                    body()
            else:
                body()

    # populate .instr bytes for extended-inst InstISA subclasses (the
    # library-reload pseudo); Bacc runs this in compile(), raw Bass doesn't
    from concourse.library_overlay import lower_extended_insts

    lower_extended_insts(nc)
    _split_multi_waits(nc)
    return nc


def make_in_maps(
    x32: np.ndarray, tokw: np.ndarray, posw: np.ndarray, sorted_mode: bool = False
):
    """Returns (in_maps, orders).  sorted_mode: slot i gathers the core's
    order[i]-th token (ascending row ids, better HBM locality); pos is
    pre-permuted to stay slot-aligned and unshard inverse-permutes."""
    in_maps, orders = [], []
    for c in range(N_CORES):
        flat = x32[:, c * POS_PER_CORE : (c + 1) * POS_PER_CORE].reshape(-1)
        if sorted_mode:
            order = np.argsort(flat, kind="stable")
            vals = flat[order]
            pc = posw[c * POS_PER_CORE + (order % POS_PER_CORE)]
        else:
            order = None
            vals = flat
            pc = posw[c * POS_PER_CORE : (c + 1) * POS_PER_CORE]
        flat16 = vals.astype(np.int16)
        # idx i -> [i%16, i//16], replicated across the 8 groups of 16
        # partitions (one replica per GPSIMD Q7 core)
        wrapped = flat16.reshape(IDX_COLS, 16).T          # [16, 128]
        xti = np.ascontiguousarray(np.tile(wrapped, (8, 1)))  # [128, 128]
        in_maps.append(
            {"xti": xti, "pos": np.ascontiguousarray(pc), "tok": tokw}
        )
        orders.append(order)
    return in_maps, orders


def unshard(results, part_major: bool = False, orders=None) -> np.ndarray:
    full = np.empty((B, T, E), dtype=np.float32)
    for c in range(N_CORES):
        oc = results[c]["out"]
        if part_major:
            # [128, 16, 512] with slot i at [i%128, i//128] -> [16, 128, 512]
            oc = oc.transpose(1, 0, 2)
        rows = oc.reshape(TOK_PER_CORE, E)
        if orders is not None and orders[c] is not None:
            # slot i holds token orders[c][i]; invert the permutation
            tok_rows = np.empty_like(rows)
            tok_rows[orders[c]] = rows
            rows = tok_rows
        full[:, c * POS_PER_CORE : (c + 1) * POS_PER_CORE, :] = rows.reshape(
            B, POS_PER_CORE, E
        )
    return full


def kernel(x: np.ndarray, tok_weight: np.ndarray, pos_weight: np.ndarray) -> np.ndarray:
    if "nc" not in _CACHE:
        _CACHE["nc"] = _build_program(sorted_mode=SORTED_MODE)
    nc = _CACHE["nc"]

    x32 = np.ascontiguousarray(np.asarray(x, dtype=np.int32))
    tokw = np.ascontiguousarray(np.asarray(tok_weight, dtype=np.float32))
    posw = np.ascontiguousarray(np.asarray(pos_weight, dtype=np.float32))

    in_maps, orders = make_in_maps(x32, tokw, posw, sorted_mode=SORTED_MODE)
    results = run_bass_kernel_spmd(nc, in_maps, core_ids=list(range(N_CORES))).results
    return unshard(results, part_major=True, orders=orders)
